# revision 23
# baseline (speedup 1.0000x reference)
"""Differential attention (B=2, N=2048, D=1024, H=8, HEAD_DIM=128) on 8 trn2
NeuronCores. Head-parallel: core h computes head h end-to-end; the heads ->
token-block reshard for the output projection is done with FOUR chunked
AllToAlls (one per pair of 512-token query blocks) so the collectives overlap
phase-B compute, and the output projection runs in four 128-token chunks as
each collective lands. Each core emits 4x128 = 512 tokens of the final output.

Layout convention on device: activations are kept feature-major ("transposed",
[feature, token]) so that matmuls contract over the partition dim without any
on-chip transposition of x. The host supplies x pre-transposed (bf16) and
transposes the output back.

Phase B is software-pipelined: the S^T matmuls for iteration k+1 are issued
before the PV matmuls of iteration k, so the PE never stalls on the Scalar
engine's exp; the steady-state rate is the ACT engine's exp throughput
(~(1024+352)/1.2 ns per key-chunk).
"""

import numpy as np

import concourse.bass as bass
import concourse.mybir as mybir
import concourse.tile as tile
from concourse.bass_utils import run_bass_kernel_spmd
from concourse.masks import make_identity
from concourse.vector_clock import ScopedClock

# ---------------------------------------------------------------- constants
B, N, D = 2, 2048, 1024
H, HD = 8, 128
DQK = HD // 2
PROJ = H * HD
T = B * N  # 4096 flattened tokens
NCORES = 8
TBLK = T // NCORES  # 512 tokens per core for the output projection
LAMBDA_INIT = 0.8 - 0.6 * float(np.exp(-0.3 * 12))
SCALE = DQK ** -0.5
EPS = 1e-6

KB = N // 128  # 16 key chunks per batch
QB = N // 512  # 4 query blocks of 512 per batch
NBLK = B * QB  # 8 query blocks total
NCOLL = 4  # chunked AllToAlls, each covering 2 query blocks

FP = mybir.dt.float32
FR = mybir.dt.float32r
BF = mybir.dt.bfloat16


# ------------------------------------------------- walrus drain workaround
# This container's walrus rejects instructions carrying >1 sync wait
# ("Too many sync wait commands"). Split multi-wait instructions.
def _split_waits(nc, inst, max_waits=1):
    si = inst.ins.sync_info
    if si is None:
        return
    waits = list(si.on_wait)
    if len(waits) <= max_waits:
        return
    si.on_wait = waits[:max_waits]
    for w in waits[max_waits:]:
        d2 = nc.sync.drain(fusable=False)
        si2 = d2.ins.sync_info
        if si2 is None:
            d2.ins.sync_info = mybir.SyncInfo(on_wait=[w], on_update=[])
        else:
            si2.on_wait = [w]


def _split_all_multiwaits(nc, max_waits=1):
    """walrus here allows only `max_waits` sync-wait per instruction. Hoist
    extra waits onto fresh NoOps inserted just before the instruction on the
    same engine (engines dispatch in order, so semantics are preserved)."""
    uid = 0
    for fn in nc.m.functions:
        for bb in fn.blocks:
            il = bb.instructions
            changed = False
            out = []
            for inst in il:
                si = inst.sync_info
                waits = list(si.on_wait) if si is not None else []
                if len(waits) > max_waits:
                    for w in waits[:-max_waits]:
                        ev = mybir.InstEventSemaphore(
                            name=f"waitsplit_{uid}",
                            sync_info=mybir.SyncInfo(on_wait=[w], on_update=[]),
                            engine=inst.engine,
                        )
                        uid += 1
                        out.append(ev)
                    si.on_wait = waits[-max_waits:]
                    if inst.sync_info is not si:
                        inst.sync_info = si
                    changed = True
                out.append(inst)
            if changed:
                bb.instructions = out


def _patched_drain_and_barrier(self, tick_clock, wait_clock):
    nc = self.nc
    drain_inst = nc.sync.drain(fusable=False)
    wait_clock.add_sem_waits(
        drain_inst.ins, ScopedClock({None: tick_clock.global_clock})
    )
    _split_waits(nc, drain_inst)
    nc.all_engine_barrier()
    assert self.sems is not None
    popped = nc._tile_sem_poison_stack.pop()
    assert popped is self._sem_poison
    nc.clear_and_free_semaphores(list(self.sems.allocated().values()))
    nc.all_engine_barrier()


tile.TileContext._drain_and_barrier = _patched_drain_and_barrier


# ---------------------------------------------------------------- program
def build_program(dbg=False, reps=1, skip_cc=False):
    nc = bass.Bass(
        "TRN2",
        target_bir_lowering=False,
        debug=False,
        enable_asserts=True,
        num_devices=NCORES,
    )

    DC = D // 128  # contraction chunks for the qkv projection
    xT = nc.dram_tensor("xT", [D, T], BF, kind="ExternalInput")
    wq = nc.dram_tensor("wq", [128, DC * HD], BF, kind="ExternalInput")
    wk = nc.dram_tensor("wk", [128, DC * HD], BF, kind="ExternalInput")
    wv = nc.dram_tensor("wv", [128, DC * HD], BF, kind="ExternalInput")
    wp = nc.dram_tensor("wp", [128, H * D], BF, kind="ExternalInput")
    lam = nc.dram_tensor("lam", [128, 1], FP, kind="ExternalInput")
    yT = nc.dram_tensor("yT", [D, TBLK], FP, kind="ExternalOutput")
    if dbg:
        d_qT = nc.dram_tensor("d_qT", [128, T], BF, kind="ExternalOutput")
        d_kT = nc.dram_tensor("d_kT", [128, T], BF, kind="ExternalOutput")
        d_va = nc.dram_tensor("d_va", [128, B * KB, HD + 1], BF, kind="ExternalOutput")
        d_U = nc.dram_tensor("d_U", [NBLK * 4, 128, 2 * (HD + 1)], FP, kind="ExternalOutput")
        d_s12 = nc.dram_tensor("d_s12", [128, 1024], FP, kind="ExternalOutput")

    with tile.TileContext(nc, num_cores=NCORES) as tc:
        with (
            tc.tile_pool(name="consts", bufs=1) as consts,
            tc.tile_pool(name="dram", bufs=1, space="DRAM") as dram,
        ):
            ident = consts.tile([128, 128], BF)
            make_identity(nc, ident)
            lam_sb = consts.tile([128, 1], FP)
            # weights + lam ride the scalar DMA queue so the x chunk loads own
            # the sync queue from the start
            nc.scalar.dma_start(lam_sb[:], lam[:])

            wq_sb = consts.tile([128, DC, HD], BF)
            wk_sb = consts.tile([128, DC, HD], BF)
            wv_sb = consts.tile([128, DC, HD], BF)
            for w_dram, w_sb in ((wq, wq_sb), (wk, wk_sb), (wv, wv_sb)):
                nc.scalar.dma_start(
                    w_sb[:], w_dram.rearrange("p (c m) -> p c m", c=DC)
                )
            wp_sb = consts.tile([128, H, D], BF)
            # preload the Exp/Ln ACT table so phase B's first exp doesn't pay
            # the table-load latency
            warm = consts.tile([128, 1], FP)
            nc.scalar.activation(
                warm[:], lam_sb[:], mybir.ActivationFunctionType.Exp
            )

            # phase C input staging: all 4 A2A chunks live here (allocated up
            # front so phase C has no pool-open WAR against phase B pools)
            aa_all = consts.tile([128, NCOLL, H, 128], BF)

            qT_b = [consts.tile([128, N], BF, name=f"qT_{b}") for b in range(B)]
            kT_b = [consts.tile([128, N], BF, name=f"kT_{b}") for b in range(B)]
            # v, per (batch, key-chunk): [key, head_dim] plus a ones column
            # (col 128) so the PV matmul also accumulates the softmax denom.
            va_b = [consts.tile([128, KB, HD + 1], BF, name=f"va_{b}") for b in range(B)]
            for b in range(B):
                nc.vector.memset(va_b[b][:, :, HD : HD + 1], 1.0)

            a2a_in = [
                dram.tile([NCORES, 128, 128], BF, name=f"a2a_in{j}")
                for j in range(NCOLL)
            ]
            a2a_out = [
                dram.tile([NCORES, 128, 128], BF, name=f"a2a_out{j}")
                for j in range(NCOLL)
            ]

            for rep in range(reps):
                # ---------------- phase A: qkv projection (feature-major) ----
                with (
                    tc.tile_pool(name="xa", bufs=2) as xa,
                    tc.tile_pool(name="pa", bufs=2, space="PSUM") as pa,
                    tc.tile_pool(name="sa", bufs=2) as sa,
                ):
                    xT_view = xT.rearrange("(c p) t -> p c t", p=128)

                    # v transposes for half X are emitted after half X+1's
                    # projection matmuls so the PE never waits on the DVE
                    # vT copy.
                    pending_vt = []

                    def flush_vt():
                        for vT, b, tb in pending_vt:
                            for j in range(4):
                                kb = (tb % QB) * 4 + j
                                vtp = pa.tile([128, 128], BF, tag="vtp",
                                              name=f"vtp_{tb}_{j}")
                                nc.tensor.transpose(
                                    vtp[:], vT[:, j * 128 : (j + 1) * 128], ident[:]
                                )
                                nc.vector.tensor_copy(va_b[b][:, kb, 0:HD], vtp[:])
                        pending_vt.clear()

                    for tp in range(T // 1024):  # 1024-token pairs
                        b = tp // 2
                        ts2 = slice(tp * 1024, (tp + 1) * 1024)
                        xx = [
                            xa.tile([128, 1024], BF, tag=f"xx{c}", name=f"xx_{tp}_{c}")
                            for c in range(DC)
                        ]
                        for c in range(DC):
                            nc.sync.dma_start(xx[c][:], xT_view[:, c, ts2])
                        for half in range(2):
                            tb = tp * 2 + half
                            hs = slice(half * 512, (half + 1) * 512)
                            bs = slice((tb % QB) * 512, (tb % QB + 1) * 512)
                            qps = pa.tile([128, 512], FP, tag="qps", name=f"qps_{tb}")
                            kps = pa.tile([128, 512], FP, tag="kps", name=f"kps_{tb}")
                            vps = pa.tile([128, 512], FP, tag="vps", name=f"vps_{tb}")
                            for c in range(DC):
                                f = dict(start=(c == 0), stop=(c == DC - 1))
                                nc.tensor.matmul(qps[:], wq_sb[:, c, :], xx[c][:, hs], **f)
                                nc.tensor.matmul(kps[:], wk_sb[:, c, :], xx[c][:, hs], **f)
                                nc.tensor.matmul(vps[:], wv_sb[:, c, :], xx[c][:, hs], **f)
                            flush_vt()
                            nc.vector.tensor_copy(qT_b[b][:, bs], qps[:])
                            nc.vector.tensor_copy(kT_b[b][:, bs], kps[:])
                            vT = sa.tile([128, 512], BF, tag="vT", name=f"vT_{tb}")
                            nc.vector.tensor_copy(vT[:], vps[:])
                            pending_vt.append((vT, b, tb))
                    flush_vt()

                    # output-projection weights: load while phase A finishes
                    nc.scalar.dma_start(
                        wp_sb[:], wp.rearrange("p (h m) -> p h m", h=H)
                    )

                # ---------------- phase B: differential attention ------------
                # iteration (blk, kb); S^T matmuls for iteration idx+1 are
                # issued ahead of the PV matmuls of iteration idx so the PE
                # stays busy while ACT computes the exp.
                blocks = [(bb, qq) for bb in range(B) for qq in range(QB)]
                iters = [(blk, kb) for blk in blocks for kb in range(KB)]
                p12_of = {}

                with (
                    tc.tile_pool(name="ps", bufs=1, space="PSUM") as ps,
                    tc.tile_pool(name="pu", bufs=1, space="PSUM") as pu,
                    tc.tile_pool(name="pp", bufs=6) as pp,
                    tc.tile_pool(name="se", bufs=2) as se,
                    tc.tile_pool(name="so", bufs=8) as so,
                ):
                    def emit_S(blk, kb):
                        b, qb = blk
                        qs = slice(qb * 512, (qb + 1) * 512)
                        ks = slice(kb * 128, (kb + 1) * 128)
                        s12 = ps.tile([128, 1024], FP, tag="s12", bufs=2,
                                      name=f"s12_{rep}_{b}_{qb}_{kb}")
                        # S^T tiles [key, query] for both q/k streams,
                        # row-packed on the PE (K=64 each, runs concurrent).
                        nc.tensor.matmul(
                            s12[:, 0:512],
                            kT_b[b][0:64, ks],
                            qT_b[b][0:64, qs],
                            start=True, stop=True,
                        )
                        nc.tensor.matmul(
                            s12[:, 512:1024],
                            kT_b[b][64:128, ks],
                            qT_b[b][64:128, qs],
                            start=True, stop=True,
                        )
                        if dbg and b == 0 and qb == 0 and kb == 0:
                            sd = pp.tile([128, 1024], FP, tag="sd")
                            nc.vector.tensor_copy(sd[:], s12[:])
                            nc.sync.dma_start(d_s12[:], sd[:])
                        p12 = pp.tile([128, 1024], BF, tag="p12",
                                      name=f"p12_{rep}_{b}_{qb}_{kb}")
                        nc.scalar.activation(
                            p12[:], s12[:], mybir.ActivationFunctionType.Exp
                        )
                        p12_of[(blk, kb)] = p12

                    U_of = {}

                    def emit_PV(blk, kb):
                        b, qb = blk
                        if kb == 0:
                            U_of[blk] = [
                                pu.tile([128, 2 * (HD + 1)], FP, tag="U", bufs=4,
                                        name=f"U_{rep}_{b}_{qb}_{i}")
                                for i in range(4)
                            ]
                        U = U_of[blk]
                        p12 = p12_of.pop((blk, kb))
                        vak = va_b[b][:, kb, :]
                        # at the last key chunk go sub-major so each U bank
                        # stops (and its SBUF copy starts) as early as possible
                        order = (
                            [(s, sub) for sub in range(4) for s in range(2)]
                            if kb == KB - 1
                            else [(s, sub) for s in range(2) for sub in range(4)]
                        )
                        for s, sub in order:
                            nc.tensor.matmul(
                                U[sub][:, s * (HD + 1) : (s + 1) * (HD + 1)],
                                p12[:, s * 512 + sub * 128 : s * 512 + (sub + 1) * 128],
                                vak,
                                start=(kb == 0 and s == 0),
                                stop=(kb == KB - 1 and s == 1),
                            )

                    def emit_epilogue(blk, streaming=False):
                        # softmax normalize, differential combine, RMSNorm;
                        # emit transposed chunks into the A2A buffers.
                        # First copy each U bank to SBUF so the PSUM slots
                        # free immediately for the next block's PV matmuls.
                        # streaming=True (last block): fully per-sub chains
                        # with per-sub rsqrt (ACT is idle by then) so the
                        # final collective triggers as early as possible.
                        b, qb = blk
                        tb = b * QB + qb
                        j = tb // 2
                        U = U_of.pop(blk)
                        ms = se.tile([128, 4], FP, tag="ms")
                        ods = []

                        def stage1(sub):
                            u = se.tile(
                                [128, 2 * (HD + 1)], FP, tag=f"usb{sub}",
                                name=f"usb_{rep}_{tb}_{sub}",
                            )
                            nc.vector.tensor_copy(u[:], U[sub][:])
                            if dbg:
                                nc.sync.dma_start(d_U[tb * 4 + sub], u[:])
                            r12 = se.tile([128, 2], FP, tag="r12")
                            nc.vector.reciprocal(
                                r12[:], u[:, HD : 2 * HD + 2 : HD + 1]
                            )
                            r2l = se.tile([128, 1], FP, tag="r2l")
                            nc.vector.tensor_mul(r2l[:], r12[:, 1:2], lam_sb[:])
                            t1 = se.tile([128, 128], FP, tag="t1")
                            t2 = se.tile([128, 128], FP, tag="t2")
                            nc.vector.tensor_scalar_mul(
                                t1[:], u[:, 0:HD], r12[:, 0:1]
                            )
                            nc.vector.tensor_scalar_mul(
                                t2[:], u[:, HD + 1 : 2 * HD + 1], r2l[:]
                            )
                            od = se.tile([128, 128], FP, tag=f"od{sub}",
                                         name=f"od_{rep}_{tb}_{sub}")
                            nc.vector.tensor_sub(od[:], t1[:], t2[:])
                            ods.append(od)
                            sq = se.tile([128, 128], FP, tag="sq")
                            nc.vector.tensor_mul(sq[:], od[:], od[:])
                            ssum = se.tile([128, 1], FP, tag="ssum")
                            nc.vector.tensor_reduce(
                                ssum[:], sq[:], mybir.AxisListType.X,
                                mybir.AluOpType.add,
                            )
                            nc.vector.tensor_scalar(
                                ms[:, sub : sub + 1], ssum[:], 1.0 / HD, EPS,
                                mybir.AluOpType.mult, mybir.AluOpType.add,
                            )

                        def rsqrt(sl):
                            rt = se.tile([128, 4], FP, tag="rt")
                            nc.scalar.activation(
                                rt[:, sl], ms[:, sl],
                                mybir.ActivationFunctionType.Ln,
                            )
                            rs = se.tile([128, 4], FP, tag="rs")
                            nc.scalar.activation(
                                rs[:, sl], rt[:, sl],
                                mybir.ActivationFunctionType.Exp, scale=-0.5,
                            )
                            return rs

                        def stage2(sub, rs):
                            on = se.tile([128, 128], BF, tag="on", bufs=4)
                            nc.vector.tensor_scalar_mul(
                                on[:], ods[sub][:], rs[:, sub : sub + 1]
                            )
                            # transpose via the DMA xbar (2-byte dtype) so
                            # the PE queue is never blocked on the epilogue
                            onT = so.tile([128, 128], BF, tag="onT")
                            nc.sync.dma_start_transpose(onT[:], on[:])
                            dest = (tb % 2) * 4 + sub
                            nc.sync.dma_start(a2a_in[j][dest], onT[:])

                        if streaming:
                            for sub in range(4):
                                stage1(sub)
                                rs = rsqrt(slice(sub, sub + 1))
                                stage2(sub, rs)
                        else:
                            for sub in range(4):
                                stage1(sub)
                            # batched rsqrt: one Ln+Exp on ACT per block,
                            # sharing the Exp table (no table-switch thrash)
                            rs = rsqrt(slice(0, 4))
                            for sub in range(4):
                                stage2(sub, rs)

                    def emit_collective(j):
                        if skip_cc:
                            nc.sync.dma_start(a2a_out[j][:], a2a_in[j][:])
                        else:
                            nc.gpsimd.collective_compute(
                                "AllToAll",
                                mybir.AluOpType.bypass,
                                replica_groups=[list(range(NCORES))],
                                ins=[a2a_in[j].opt()],
                                outs=[a2a_out[j].opt()],
                            )
                        # stage chunk j for phase C right behind the
                        # collective on the (otherwise idle) gpsimd queue —
                        # FIFO with the collective, blocks no other engine
                        nc.gpsimd.dma_start(
                            aa_all[:, j], a2a_out[j].rearrange("h p t -> p h t")
                        )

                    emit_S(*iters[0])
                    for idx, (blk, kb) in enumerate(iters):
                        if idx + 1 < len(iters):
                            emit_S(*iters[idx + 1])
                        emit_PV(blk, kb)
                        if kb == KB - 1:
                            emit_epilogue(blk, streaming=(idx == len(iters) - 1))
                            tb = blk[0] * QB + blk[1]
                            if tb % 2 == 1:
                                emit_collective(tb // 2)

                if dbg:
                    for b in range(B):
                        nc.sync.dma_start(d_qT[:, b * N : (b + 1) * N], qT_b[b][:])
                        nc.sync.dma_start(d_kT[:, b * N : (b + 1) * N], kT_b[b][:])
                        nc.sync.dma_start(d_va[:, b * KB : (b + 1) * KB, :], va_b[b][:])

                # ---------------- phase C: output projection per A2A chunk ---
                # aa loads + yT writes ride the scalar (ACT) DMA queue, which
                # is idle once the exps are done; loads are prefetched one
                # chunk ahead, one strided DMA per chunk.
                yT_view = yT.rearrange("(c p) t -> p c t", p=128)
                with (
                    tc.tile_pool(name="pc", bufs=2, space="PSUM") as pc,
                    tc.tile_pool(name="sy", bufs=2) as sy,
                ):
                    for j in range(NCOLL):
                        yo = sy.tile([128, D // 128, 128], FP, tag="yo")
                        for oc in range(D // 128):
                            yps = pc.tile([128, 128], FP, tag="yps")
                            for hh in range(H):
                                nc.tensor.matmul(
                                    yps[:],
                                    wp_sb[:, hh, oc * 128 : (oc + 1) * 128],
                                    aa_all[:, j, hh, :],
                                    start=(hh == 0),
                                    stop=(hh == H - 1),
                                )
                            nc.vector.tensor_copy(yo[:, oc], yps[:])
                        nc.sync.dma_start(
                            yT_view[:, :, j * 128 : (j + 1) * 128], yo[:]
                        )

    _split_all_multiwaits(nc)
    return nc


_PROGRAM = None


def _get_program():
    global _PROGRAM
    if _PROGRAM is None:
        _PROGRAM = build_program()
    return _PROGRAM


# ---------------------------------------------------------------- host side
def _prep_in_maps(x, w_qkv, w_proj, lambda_q1, lambda_k1, lambda_q2, lambda_k2,
                  rms_weight):
    import ml_dtypes

    x = np.asarray(x, dtype=np.float32)
    w_qkv = np.asarray(w_qkv, dtype=np.float32)
    w_proj = np.asarray(w_proj, dtype=np.float32)
    xT = np.ascontiguousarray(x.reshape(T, D).T).astype(ml_dtypes.bfloat16)
    lam_val = (
        float(np.exp(np.sum(np.asarray(lambda_q1, np.float64) * np.asarray(lambda_k1, np.float64))))
        - float(np.exp(np.sum(np.asarray(lambda_q2, np.float64) * np.asarray(lambda_k2, np.float64))))
        + LAMBDA_INIT
    )
    lam_arr = np.full((128, 1), lam_val, dtype=np.float32)
    # fold rms_weight and (1 - lambda_init) into the output projection rows
    rw = np.asarray(rms_weight, np.float32)
    wp_full = np.ascontiguousarray(
        w_proj * np.tile(rw, H)[:, None] * np.float32(1.0 - LAMBDA_INIT)
    )
    # device-friendly layouts: weights arranged so each DMA descriptor is a
    # long contiguous run per partition
    def chunked(w):  # [D, HD] -> [128, DC*HD] with [p, c*HD+m] = w[c*128+p, m]
        dc = D // 128
        return np.ascontiguousarray(
            w.reshape(dc, 128, HD).transpose(1, 0, 2).reshape(128, dc * HD)
        ).astype(ml_dtypes.bfloat16)

    wp_dev = np.ascontiguousarray(
        wp_full.reshape(H, 128, D).transpose(1, 0, 2).reshape(128, H * D)
    ).astype(ml_dtypes.bfloat16)
    in_maps = []
    for h in range(NCORES):
        hs = slice(h * HD, (h + 1) * HD)
        in_maps.append(
            {
                "xT": xT,
                "wq": chunked(np.ascontiguousarray(w_qkv[:, hs]) * np.float32(SCALE)),
                "wk": chunked(w_qkv[:, PROJ + h * HD : PROJ + (h + 1) * HD]),
                "wv": chunked(w_qkv[:, 2 * PROJ + h * HD : 2 * PROJ + (h + 1) * HD]),
                "wp": wp_dev,
                "lam": lam_arr,
            }
        )
    return in_maps


def _assemble(results):
    y = np.empty((T, D), dtype=np.float32)
    for c in range(NCORES):
        yt = results[c]["yT"]  # [D, 4*128], chunk j = tokens of collective j
        for j in range(NCOLL):
            tb = 2 * j + c // 4
            t0 = tb * 512 + (c % 4) * 128
            y[t0 : t0 + 128, :] = yt[:, j * 128 : (j + 1) * 128].T
    return y.reshape(B, N, D)


def kernel(x, w_qkv, w_proj, lambda_q1, lambda_k1, lambda_q2, lambda_k2,
           rms_weight):
    nc = _get_program()
    in_maps = _prep_in_maps(
        x, w_qkv, w_proj, lambda_q1, lambda_k1, lambda_q2, lambda_k2, rms_weight
    )
    res = run_bass_kernel_spmd(nc, in_maps, list(range(NCORES)))
    return _assemble(res.results)
